# revision 39
# baseline (speedup 1.0000x reference)
"""BERT self-attention (B=16, S=512, H=768, NH=12, HD=64) on 8 trn2 NeuronCores.

Sharding: data-parallel over batch — 2 batches per core. Each core runs an
identical Bass/Tile program on its batch shard and produces its slice of
(ctx, probs); the host gathers along batch.

Device-side formulation, per (batch, head-pair):
  The QK^T projection emits Q^T/K^T in [HD, S] layout with the even head of
  each pair on partitions 0-63 and the odd head on partitions 64-127. Score
  matmuls for the two heads are emitted adjacently: they occupy disjoint PE
  row groups (rows 0-63 vs 64-127) and execute concurrently.
  S^T[k,q] tiles (k on partitions) -> exp (ScalarE, scale=1/8 folded in, no
  max-subtraction: scores here are in [-3, 3]) -> E^T.
  ctx^T[d,q] accumulates V'[k, d|1] E^T[k,q] over k-tiles where V' carries a
  trailing ones column, so row 64 of the accumulator is the softmax
  denominator row sum_k E^T[k,q].
  S[q,k] tiles -> exp -> P = E * (1/sum) in place -> probs out.
  ctx^T is transposed back per q-tile on the PE and normalized by 1/sum
  during PSUM evacuation.
"""

import threading
from contextlib import ExitStack

import numpy as np

import concourse.bass as bass
import concourse.mybir as mybir
import concourse.tile as tile
from concourse import bacc
from concourse.bass_utils import run_bass_kernel_spmd
from concourse.masks import make_identity

B, S, H = 16, 512, 768
NH, HD = 12, 64
NCORES = 8
BPC = B // NCORES  # batches per core
F32 = mybir.dt.float32
F32R = mybir.dt.float32r
SCALE = 1.0 / 8.0  # 1/sqrt(HD)


def build_program():
    nc = bacc.Bacc(
        "TRN2",
        target_bir_lowering=False,
        debug=False,
        num_devices=NCORES,
    )

    xt = nc.dram_tensor("xt", [BPC, H, S], F32R, kind="ExternalInput").ap()
    wqkt = nc.dram_tensor("wqkt", [H, 2 * H], F32R, kind="ExternalInput").ap()
    wvt = nc.dram_tensor("wvt", [H, H], F32R, kind="ExternalInput").ap()
    bqk = nc.dram_tensor("bqk", [2 * H], F32, kind="ExternalInput").ap()
    bv = nc.dram_tensor("bv", [H], F32, kind="ExternalInput").ap()
    probs = nc.dram_tensor("probs", [BPC, NH, S, S], F32, kind="ExternalOutput").ap()
    ctxo = nc.dram_tensor("ctxo", [BPC, S, H], F32, kind="ExternalOutput").ap()
    sums_dram = nc.dram_tensor("sums_scratch", [BPC, NH, S], F32).ap()

    with tile.TileContext(nc) as tc, ExitStack() as ctx:
        singles = ctx.enter_context(tc.tile_pool(name="singles", bufs=1))
        xtp = ctx.enter_context(tc.tile_pool(name="xtp", bufs=2))
        qkp = ctx.enter_context(tc.tile_pool(name="qkp", bufs=2))
        vp = ctx.enter_context(tc.tile_pool(name="vp", bufs=2))
        etp = ctx.enter_context(tc.tile_pool(name="etp", bufs=6))
        pp = ctx.enter_context(tc.tile_pool(name="pp", bufs=4))
        ctxtp = ctx.enter_context(tc.tile_pool(name="ctxtp", bufs=2))
        ctxfp = ctx.enter_context(tc.tile_pool(name="ctxfp", bufs=2))
        smallp = ctx.enter_context(tc.tile_pool(name="smallp", bufs=4))
        pspool = ctx.enter_context(tc.tile_pool(name="pspool", bufs=3, space="PSUM"))
        ctxps = ctx.enter_context(tc.tile_pool(name="ctxps", bufs=1, space="PSUM"))
        trps = ctx.enter_context(tc.tile_pool(name="trps", bufs=1, space="PSUM"))

        # --- persistent tiles; DMAs are ordered so compute starts early ---
        wqk_sb = singles.tile([128, 6, 2 * H], F32R)
        wv_sb = singles.tile([128, 6, H], F32R)
        wqkt_r = wqkt.rearrange("(t p) o -> p t o", p=128)
        wvt_r = wvt.rearrange("(t p) o -> p t o", p=128)
        bqk_sb = singles.tile([128, 12], F32)
        bv_sb = singles.tile([128, H], F32)
        ident = singles.tile([128, 128], F32)
        make_identity(nc, ident)
        identr = singles.tile([128, 128], F32R)
        nc.vector.tensor_copy(identr, ident)

        def proj_start(b):
            """Allocate batch-b projection tiles and start the xt loads.

            qk_sb [128, 12, 512]: slot 2j = Q^T for heads (2j, 2j+1) stacked
            on partitions (0-63, 64-127); slot 2j+1 = same for K^T.
            v_sb [128, 4, 12, 65]: (t-tile, head) with a trailing ones col.
            """
            xt_sb = xtp.tile([128, 6, S], F32R, tag="xt_sb", name=f"xt_sb_{b}")
            xt_r = xt[b].rearrange("(t p) s -> p t s", p=128)
            for t in range(6):
                nc.sync.dma_start(out=xt_sb[:, t, :], in_=xt_r[:, t, :])
            qk_sb = qkp.tile([128, 12, S], F32R, tag="qk_sb", name=f"qk_sb_{b}")
            v_sb = vp.tile([128, 4, 12, 65], F32R, tag="v_sb", name=f"v_sb_{b}")
            nc.vector.memset(v_sb[:, :, :, 64:65].bitcast(F32), 1.0)
            return xt_sb, qk_sb, v_sb

        def proj_qk_unit(c, xt_sb, qk_sb):
            ps = pspool.tile([128, 1024], F32, tag="ps", name=f"ps_qk_{c}", uniquify=True)
            for half in range(2):
                s_slot = 2 * c + half
                for t in range(6):
                    nc.tensor.matmul(
                        ps[:, 512 * half : 512 * half + 512],
                        wqk_sb[:, t, bass.ts(s_slot, 128)],
                        xt_sb[:, t, :],
                        start=(t == 0),
                        stop=(t == 5),
                    )
            for half in range(2):
                s_slot = 2 * c + half
                nc.vector.tensor_scalar_add(
                    qk_sb[:, s_slot, :],
                    ps[:, 512 * half : 512 * half + 512],
                    bqk_sb[:, s_slot : s_slot + 1],
                )

        def proj_v_unit(tt, xt_sb, v_sb):
            ps = pspool.tile([128, 1024], F32, tag="ps", name=f"ps_v_{tt}")
            for chunk in range(2):  # o halves of 384, bank-aligned at 0/512
                for t in range(6):
                    nc.tensor.matmul(
                        ps[:, 512 * chunk : 512 * chunk + 384],
                        xt_sb[:, t, bass.ts(tt, 128)],
                        wv_sb[:, t, 384 * chunk : 384 * chunk + 384],
                        start=(t == 0),
                        stop=(t == 5),
                    )
            for chunk in range(2):  # 384 cols = 6 heads
                nc.vector.tensor_add(
                    v_sb[:, tt, 6 * chunk : 6 * chunk + 6, 0:64],
                    ps[:, 512 * chunk : 512 * chunk + 384].rearrange(
                        "p (h d) -> p h d", d=64
                    ),
                    bv_sb[:, 384 * chunk : 384 * chunk + 384].rearrange(
                        "p (h d) -> p h d", d=64
                    ),
                )

        def proj_all(b):
            xt_sb, qk_sb, v_sb = proj_start(b)
            for c in range(6):
                proj_qk_unit(c, xt_sb, qk_sb)
            for tt in range(4):
                proj_v_unit(tt, xt_sb, v_sb)
            return qk_sb, v_sb

        def attn_pair_scores(b, j, qk_sb, v_sb):
            """Heads (2j, 2j+1): score matmuls for the two heads are emitted
            adjacently on disjoint PE row groups so they run concurrently.
            Returns a closure emitting the ctx/normalize/output stage."""
            h0, h1 = 2 * j, 2 * j + 1
            qt = {0: qk_sb[0:64, 2 * j, :], 1: qk_sb[64:128, 2 * j, :]}
            kt = {0: qk_sb[0:64, 2 * j + 1, :], 1: qk_sb[64:128, 2 * j + 1, :]}

            # S^T: chunk tile [128, 1024] = k-tile x {h0, h1}
            et_tiles = []
            for k_tile in range(4):
                st_ps = pspool.tile([128, 1024], F32, tag="ps")
                for p in range(2):
                    nc.tensor.matmul(
                        st_ps[:, 512 * p : 512 * p + 512],
                        kt[p][:, bass.ts(k_tile, 128)],
                        qt[p],
                        start=True,
                        stop=True,
                    )
                et_sb = etp.tile([128, 1024], F32R, tag="et")
                nc.scalar.activation(
                    et_sb, st_ps, mybir.ActivationFunctionType.Exp, scale=SCALE
                )
                et_tiles.append(et_sb)

            def tail():
                emit_ctx_tail(b, j, qt, kt, v_sb, et_tiles)
            return tail

        def emit_ctx_tail(b, j, qt, kt, v_sb, et_tiles):
            h0, h1 = 2 * j, 2 * j + 1
            # ctx^T + denominators per head ([65, 512] accumulator)
            cps = {}
            for p, h in ((0, h0), (1, h1)):
                cps[p] = ctxps.tile([128, 512], F32, tag="cps", name=f"cps_{b}_{j}_{p}")
                for k_tile in range(4):
                    nc.tensor.matmul(
                        cps[p][0:65, :],
                        v_sb[:, k_tile, h, :],
                        et_tiles[k_tile][:, 512 * p : 512 * p + 512],
                        start=(k_tile == 0),
                        stop=(k_tile == 3),
                    )

            # S side: chunk tile [128, 1024] = q-tile x {h0, h1}
            s_chunks = []
            for m in range(4):
                s_ps = pspool.tile([128, 1024], F32, tag="ps")
                for p in range(2):
                    nc.tensor.matmul(
                        s_ps[:, 512 * p : 512 * p + 512],
                        qt[p][:, bass.ts(m, 128)],
                        kt[p],
                        start=True,
                        stop=True,
                    )
                p_sb = pp.tile([128, 1024], F32, tag="pe_chunk")
                nc.scalar.activation(
                    p_sb, s_ps, mybir.ActivationFunctionType.Exp, scale=SCALE
                )
                s_chunks.append(p_sb)

            for p, h in ((0, h0), (1, h1)):
                # evacuate ctx^T plus denominator row in one copy
                ctxt_sb = ctxtp.tile([65, 512], F32)
                nc.vector.tensor_copy(ctxt_sb, cps[p][0:65, :])

                # denominators [1,512] -> [128,4] via DRAM bounce
                nc.sync.dma_start(
                    out=sums_dram[b, h].unsqueeze(0), in_=ctxt_sb[64:65, :]
                )
                sums_sb = smallp.tile([128, 4], F32, tag="sums")
                nc.sync.dma_start(
                    out=sums_sb, in_=sums_dram[b, h].rearrange("(m p) -> p m", p=128)
                )
                r_sb = smallp.tile([128, 4], F32, tag="r")
                nc.vector.reciprocal(r_sb, sums_sb)

                # ctx^T -> ctx via PE transpose, normalized on evacuation
                ctxf_sb = ctxfp.tile([128, 4, HD], F32)
                tps = trps.tile([128, 256], F32)
                for m in range(4):
                    nc.tensor.transpose(
                        tps[:, bass.ts(m, 64)],
                        ctxt_sb[0:64, bass.ts(m, 128)],
                        ident[0:64, 0:64],
                    )
                for m in range(4):
                    nc.vector.tensor_scalar_mul(
                        ctxf_sb[:, m, :], tps[:, bass.ts(m, 64)], r_sb[:, m : m + 1]
                    )
                nc.sync.dma_start(
                    out=ctxo[b].rearrange("(m p) o -> p m o", p=128)[
                        :, :, HD * h : HD * h + HD
                    ],
                    in_=ctxf_sb,
                )

                # normalize P in place and store probs
                for m in range(4):
                    half = s_chunks[m][:, 512 * p : 512 * p + 512]
                    nc.vector.tensor_scalar_mul(half, half, r_sb[:, m : m + 1])
                    nc.sync.dma_start(
                        out=probs[b, h, bass.ts(m, 128), :], in_=half
                    )

        # batch-0 activations first, then QK weights, so the first QK
        # projection matmuls start as soon as those DMAs land.
        xt0, qk0, v0 = proj_start(0)
        for t in range(6):
            nc.sync.dma_start(out=wqk_sb[:, t, :], in_=wqkt_r[:, t, :])
        nc.sync.dma_start(out=bqk_sb, in_=bqk.rearrange("(s p) -> p s", p=128))
        for c in range(6):
            proj_qk_unit(c, xt0, qk0)
        for t in range(6):
            nc.sync.dma_start(out=wv_sb[:, t, :], in_=wvt_r[:, t, :])
        nc.sync.dma_start(out=bv_sb, in_=bv.unsqueeze(0).to_broadcast([128, H]))
        for tt in range(4):
            proj_v_unit(tt, xt0, v0)

        # batch 1's projection units are interleaved between batch 0's
        # attention pairs: they fill PE gaps while ScalarE works on exps.
        xt1, qk1, v1 = proj_start(1)
        units = [lambda c=c: proj_qk_unit(c, xt1, qk1) for c in range(6)]
        units += [lambda tt=tt: proj_v_unit(tt, xt1, v1) for tt in range(4)]
        for b, qk_b, v_b in ((0, qk0, v0), (1, qk1, v1)):
            for j in range(NH // 2):
                tail = attn_pair_scores(b, j, qk_b, v_b)
                tail()
                if b == 0:
                    for u in units[2 * j : 2 * j + 2]:
                        u()

    nc.compile()
    return nc


_CACHE = {}
_LOCK = threading.Lock()

# test-harness hooks: set TRACE=True before calling kernel() to profile; the
# BassKernelResults of the last run lands in LAST_RESULT.
TRACE = False
LAST_RESULT = None


def _get_nc():
    with _LOCK:
        if "nc" not in _CACHE:
            _CACHE["nc"] = build_program()
        return _CACHE["nc"]


def _numpy_fallback(hidden_state, attention_mask, Wq, bq, Wk, bk, Wv, bv):
    x = hidden_state.astype(np.float32)
    q = (x @ Wq.T + bq).reshape(B, S, NH, HD).transpose(0, 2, 1, 3)
    k = (x @ Wk.T + bk).reshape(B, S, NH, HD).transpose(0, 2, 1, 3)
    v = (x @ Wv.T + bv).reshape(B, S, NH, HD).transpose(0, 2, 1, 3)
    scores = np.einsum("bhqd,bhkd->bhqk", q, k).astype(np.float32) * np.float32(SCALE)
    scores = scores + ((1.0 - attention_mask)[:, None, None, :] * -10000.0).astype(
        np.float32
    )
    m = scores.max(-1, keepdims=True)
    e = np.exp(scores - m, dtype=np.float32)
    p = e / e.sum(-1, keepdims=True, dtype=np.float32)
    ctx = np.einsum("bhqk,bhkd->bhqd", p, v).astype(np.float32)
    ctx = ctx.transpose(0, 2, 1, 3).reshape(B, S, NH * HD)
    return ctx.astype(np.float32), p.astype(np.float32)


def kernel(hidden_state, attention_mask, Wq, bq, Wk, bk, Wv, bv):
    hidden_state = np.ascontiguousarray(np.asarray(hidden_state, dtype=np.float32))
    attention_mask = np.asarray(attention_mask, dtype=np.float32)
    Wq = np.asarray(Wq, dtype=np.float32)
    Wk = np.asarray(Wk, dtype=np.float32)
    Wv = np.asarray(Wv, dtype=np.float32)
    bq = np.asarray(bq, dtype=np.float32)
    bk = np.asarray(bk, dtype=np.float32)
    bv = np.asarray(bv, dtype=np.float32)

    # The device program folds the (all-ones per the problem spec) attention
    # mask out entirely; handle the general case exactly on the host.
    if not np.all(attention_mask == 1.0):
        return _numpy_fallback(hidden_state, attention_mask, Wq, bq, Wk, bk, Wv, bv)

    # Interleave Wq/Wk in 128-row blocks so each projection output M-tile
    # holds [Q^T head pair; K^T head pair] on matching partition ranges.
    wqk = np.empty((2 * H, H), dtype=np.float32)
    bqk = np.empty((2 * H,), dtype=np.float32)
    for jj in range(6):
        wqk[256 * jj : 256 * jj + 128] = Wq[128 * jj : 128 * jj + 128]
        wqk[256 * jj + 128 : 256 * jj + 256] = Wk[128 * jj : 128 * jj + 128]
        bqk[256 * jj : 256 * jj + 128] = bq[128 * jj : 128 * jj + 128]
        bqk[256 * jj + 128 : 256 * jj + 256] = bk[128 * jj : 128 * jj + 128]
    wqkt = np.ascontiguousarray(wqk.T)
    wvt = np.ascontiguousarray(Wv.T)

    nc = _get_nc()
    in_maps = []
    for c in range(NCORES):
        xs = hidden_state[BPC * c : BPC * c + BPC]
        in_maps.append(
            {
                "xt": np.ascontiguousarray(xs.transpose(0, 2, 1)),
                "wqkt": wqkt,
                "wvt": wvt,
                "bqk": bqk,
                "bv": bv,
            }
        )
    res = run_bass_kernel_spmd(nc, in_maps, core_ids=list(range(NCORES)), trace=TRACE)
    global LAST_RESULT
    LAST_RESULT = res
    ctx = np.concatenate([r["ctxo"] for r in res.results], axis=0)
    probs = np.concatenate([r["probs"] for r in res.results], axis=0)
    return ctx.astype(np.float32, copy=False), probs.astype(np.float32, copy=False)


# revision 40
# speedup vs baseline: 1.0386x; 1.0386x over previous
"""BERT self-attention (B=16, S=512, H=768, NH=12, HD=64) on 8 trn2 NeuronCores.

Sharding: data-parallel over batch — 2 batches per core. Each core runs an
identical Bass/Tile program on its batch shard and produces its slice of
(ctx, probs); the host gathers along batch.

Device-side formulation, per (batch, head-pair):
  The QK^T projection emits Q^T/K^T in [HD, S] layout with the even head of
  each pair on partitions 0-63 and the odd head on partitions 64-127. Score
  matmuls for the two heads are emitted adjacently: they occupy disjoint PE
  row groups (rows 0-63 vs 64-127) and execute concurrently.
  S^T[k,q] tiles (k on partitions) -> exp (ScalarE, scale=1/8 folded in, no
  max-subtraction: scores here are in [-3, 3]) -> E^T.
  ctx^T[d,q] accumulates V'[k, d|1] E^T[k,q] over k-tiles where V' carries a
  trailing ones column, so row 64 of the accumulator is the softmax
  denominator row sum_k E^T[k,q].
  S[q,k] tiles -> exp -> P = E * (1/sum) in place -> probs out.
  ctx^T is transposed back per q-tile on the PE and normalized by 1/sum
  during PSUM evacuation.
"""

import threading
from contextlib import ExitStack

import numpy as np

import concourse.bass as bass
import concourse.mybir as mybir
import concourse.tile as tile
from concourse import bacc
from concourse.bass_utils import run_bass_kernel_spmd
from concourse.masks import make_identity

B, S, H = 16, 512, 768
NH, HD = 12, 64
NCORES = 8
BPC = B // NCORES  # batches per core
F32 = mybir.dt.float32
F32R = mybir.dt.float32r
SCALE = 1.0 / 8.0  # 1/sqrt(HD)


def build_program():
    nc = bacc.Bacc(
        "TRN2",
        target_bir_lowering=False,
        debug=False,
        num_devices=NCORES,
    )

    xt = nc.dram_tensor("xt", [BPC, H, S], F32R, kind="ExternalInput").ap()
    wqkt = nc.dram_tensor("wqkt", [H, 2 * H], F32R, kind="ExternalInput").ap()
    wvt = nc.dram_tensor("wvt", [H, H], F32R, kind="ExternalInput").ap()
    bqk = nc.dram_tensor("bqk", [2 * H], F32, kind="ExternalInput").ap()
    bv = nc.dram_tensor("bv", [H], F32, kind="ExternalInput").ap()
    probs = nc.dram_tensor("probs", [BPC, NH, S, S], F32, kind="ExternalOutput").ap()
    ctxo = nc.dram_tensor("ctxo", [BPC, S, H], F32, kind="ExternalOutput").ap()
    sums_dram = nc.dram_tensor("sums_scratch", [BPC, NH, S], F32).ap()

    with tile.TileContext(nc) as tc, ExitStack() as ctx:
        singles = ctx.enter_context(tc.tile_pool(name="singles", bufs=1))
        xtp = ctx.enter_context(tc.tile_pool(name="xtp", bufs=2))
        qkp = ctx.enter_context(tc.tile_pool(name="qkp", bufs=2))
        vp = ctx.enter_context(tc.tile_pool(name="vp", bufs=2))
        etp = ctx.enter_context(tc.tile_pool(name="etp", bufs=5))
        pp = ctx.enter_context(tc.tile_pool(name="pp", bufs=4))
        ctxtp = ctx.enter_context(tc.tile_pool(name="ctxtp", bufs=2))
        ctxfp = ctx.enter_context(tc.tile_pool(name="ctxfp", bufs=2))
        smallp = ctx.enter_context(tc.tile_pool(name="smallp", bufs=4))
        pspool = ctx.enter_context(tc.tile_pool(name="pspool", bufs=3, space="PSUM"))
        ctxps = ctx.enter_context(tc.tile_pool(name="ctxps", bufs=1, space="PSUM"))
        trps = ctx.enter_context(tc.tile_pool(name="trps", bufs=1, space="PSUM"))

        # --- persistent tiles; DMAs are ordered so compute starts early ---
        wqk_sb = singles.tile([128, 6, 2 * H], F32R)
        wv_sb = singles.tile([128, 6, H], F32R)
        wqkt_r = wqkt.rearrange("(t p) o -> p t o", p=128)
        wvt_r = wvt.rearrange("(t p) o -> p t o", p=128)
        bqk_sb = singles.tile([128, 12], F32)
        bv_sb = singles.tile([128, H], F32)
        ident = singles.tile([128, 128], F32)
        make_identity(nc, ident)
        identr = singles.tile([128, 128], F32R)
        nc.vector.tensor_copy(identr, ident)

        def proj_start(b):
            """Allocate batch-b projection tiles and start the xt loads.

            qk_sb [128, 12, 512]: slot 2j = Q^T for heads (2j, 2j+1) stacked
            on partitions (0-63, 64-127); slot 2j+1 = same for K^T.
            v_sb [128, 4, 12, 65]: (t-tile, head) with a trailing ones col.
            """
            xt_sb = xtp.tile([128, 6, S], F32R, tag="xt_sb", name=f"xt_sb_{b}")
            xt_r = xt[b].rearrange("(t p) s -> p t s", p=128)
            for t in range(6):
                nc.sync.dma_start(out=xt_sb[:, t, :], in_=xt_r[:, t, :])
            qk_sb = qkp.tile([128, 12, S], F32R, tag="qk_sb", name=f"qk_sb_{b}")
            v_sb = vp.tile([128, 4, 12, 65], F32R, tag="v_sb", name=f"v_sb_{b}")
            nc.vector.memset(v_sb[:, :, :, 64:65].bitcast(F32), 1.0)
            return xt_sb, qk_sb, v_sb

        def proj_qk_unit(c, xt_sb, qk_sb):
            ps = pspool.tile([128, 1024], F32, tag="ps", name=f"ps_qk_{c}", uniquify=True)
            for half in range(2):
                s_slot = 2 * c + half
                for t in range(6):
                    nc.tensor.matmul(
                        ps[:, 512 * half : 512 * half + 512],
                        wqk_sb[:, t, bass.ts(s_slot, 128)],
                        xt_sb[:, t, :],
                        start=(t == 0),
                        stop=(t == 5),
                    )
            for half in range(2):
                s_slot = 2 * c + half
                nc.vector.tensor_scalar_add(
                    qk_sb[:, s_slot, :],
                    ps[:, 512 * half : 512 * half + 512],
                    bqk_sb[:, s_slot : s_slot + 1],
                )

        def proj_v_unit(tt, xt_sb, v_sb):
            ps = pspool.tile([128, 1024], F32, tag="ps", name=f"ps_v_{tt}")
            for chunk in range(2):  # o halves of 384, bank-aligned at 0/512
                for t in range(6):
                    nc.tensor.matmul(
                        ps[:, 512 * chunk : 512 * chunk + 384],
                        xt_sb[:, t, bass.ts(tt, 128)],
                        wv_sb[:, t, 384 * chunk : 384 * chunk + 384],
                        start=(t == 0),
                        stop=(t == 5),
                    )
            for chunk in range(2):  # 384 cols = 6 heads
                nc.vector.tensor_add(
                    v_sb[:, tt, 6 * chunk : 6 * chunk + 6, 0:64],
                    ps[:, 512 * chunk : 512 * chunk + 384].rearrange(
                        "p (h d) -> p h d", d=64
                    ),
                    bv_sb[:, 384 * chunk : 384 * chunk + 384].rearrange(
                        "p (h d) -> p h d", d=64
                    ),
                )

        def proj_all(b):
            xt_sb, qk_sb, v_sb = proj_start(b)
            for c in range(6):
                proj_qk_unit(c, xt_sb, qk_sb)
            for tt in range(4):
                proj_v_unit(tt, xt_sb, v_sb)
            return qk_sb, v_sb

        def attn_pair_scores(b, j, qk_sb, v_sb):
            """Heads (2j, 2j+1): score matmuls for the two heads are emitted
            adjacently on disjoint PE row groups so they run concurrently.
            Returns a closure emitting the ctx/normalize/output stage."""
            h0, h1 = 2 * j, 2 * j + 1
            qt = {0: qk_sb[0:64, 2 * j, :], 1: qk_sb[64:128, 2 * j, :]}
            kt = {0: qk_sb[0:64, 2 * j + 1, :], 1: qk_sb[64:128, 2 * j + 1, :]}

            # S^T: chunk tile [128, 1024] = k-tile x {h0, h1}
            et_tiles = []
            for k_tile in range(4):
                st_ps = pspool.tile([128, 1024], F32, tag="ps")
                for p in range(2):
                    nc.tensor.matmul(
                        st_ps[:, 512 * p : 512 * p + 512],
                        kt[p][:, bass.ts(k_tile, 128)],
                        qt[p],
                        start=True,
                        stop=True,
                    )
                et_sb = etp.tile([128, 1024], F32R, tag="et")
                nc.scalar.activation(
                    et_sb, st_ps, mybir.ActivationFunctionType.Exp, scale=SCALE
                )
                et_tiles.append(et_sb)

            def tail():
                emit_ctx_tail(b, j, qt, kt, v_sb, et_tiles)
            return tail

        def emit_ctx_tail(b, j, qt, kt, v_sb, et_tiles):
            h0, h1 = 2 * j, 2 * j + 1
            # ctx^T + denominators per head ([65, 512] accumulator)
            cps = {}
            for p, h in ((0, h0), (1, h1)):
                cps[p] = ctxps.tile([128, 512], F32, tag="cps", name=f"cps_{b}_{j}_{p}")
                for k_tile in range(4):
                    nc.tensor.matmul(
                        cps[p][0:65, :],
                        v_sb[:, k_tile, h, :],
                        et_tiles[k_tile][:, 512 * p : 512 * p + 512],
                        start=(k_tile == 0),
                        stop=(k_tile == 3),
                    )

            # S side: chunk tile [128, 1024] = q-tile x {h0, h1}
            s_chunks = []
            for m in range(4):
                s_ps = pspool.tile([128, 1024], F32, tag="ps")
                for p in range(2):
                    nc.tensor.matmul(
                        s_ps[:, 512 * p : 512 * p + 512],
                        qt[p][:, bass.ts(m, 128)],
                        kt[p],
                        start=True,
                        stop=True,
                    )
                p_sb = pp.tile([128, 1024], F32, tag="pe_chunk")
                nc.scalar.activation(
                    p_sb, s_ps, mybir.ActivationFunctionType.Exp, scale=SCALE
                )
                s_chunks.append(p_sb)

            for p, h in ((0, h0), (1, h1)):
                # evacuate ctx^T plus denominator row in one copy
                ctxt_sb = ctxtp.tile([65, 512], F32)
                nc.vector.tensor_copy(ctxt_sb, cps[p][0:65, :])

                # denominators [1,512] -> [128,4] via DRAM bounce
                nc.sync.dma_start(
                    out=sums_dram[b, h].unsqueeze(0), in_=ctxt_sb[64:65, :]
                )
                sums_sb = smallp.tile([128, 4], F32, tag="sums")
                nc.sync.dma_start(
                    out=sums_sb, in_=sums_dram[b, h].rearrange("(m p) -> p m", p=128)
                )
                r_sb = smallp.tile([128, 4], F32, tag="r")
                nc.vector.reciprocal(r_sb, sums_sb)

                # ctx^T -> ctx via PE transpose, normalized on evacuation
                ctxf_sb = ctxfp.tile([128, 4, HD], F32)
                tps = trps.tile([128, 256], F32)
                for m in range(4):
                    nc.tensor.transpose(
                        tps[:, bass.ts(m, 64)],
                        ctxt_sb[0:64, bass.ts(m, 128)],
                        ident[0:64, 0:64],
                    )
                for m in range(4):
                    nc.vector.tensor_scalar_mul(
                        ctxf_sb[:, m, :], tps[:, bass.ts(m, 64)], r_sb[:, m : m + 1]
                    )
                nc.sync.dma_start(
                    out=ctxo[b].rearrange("(m p) o -> p m o", p=128)[
                        :, :, HD * h : HD * h + HD
                    ],
                    in_=ctxf_sb,
                )

                # normalize P in place and store probs
                for m in range(4):
                    half = s_chunks[m][:, 512 * p : 512 * p + 512]
                    nc.vector.tensor_scalar_mul(half, half, r_sb[:, m : m + 1])
                    nc.sync.dma_start(
                        out=probs[b, h, bass.ts(m, 128), :], in_=half
                    )

        # batch-0 activations first, then QK weights, so the first QK
        # projection matmuls start as soon as those DMAs land.
        xt0, qk0, v0 = proj_start(0)
        for t in range(6):
            nc.sync.dma_start(out=wqk_sb[:, t, :], in_=wqkt_r[:, t, :])
        nc.sync.dma_start(out=bqk_sb, in_=bqk.rearrange("(s p) -> p s", p=128))
        for c in range(6):
            proj_qk_unit(c, xt0, qk0)
        for t in range(6):
            nc.sync.dma_start(out=wv_sb[:, t, :], in_=wvt_r[:, t, :])
        nc.sync.dma_start(out=bv_sb, in_=bv.unsqueeze(0).to_broadcast([128, H]))
        for tt in range(4):
            proj_v_unit(tt, xt0, v0)

        # batch 1's projection units are interleaved between batch 0's
        # attention pairs: they fill PE gaps while ScalarE works on exps.
        xt1, qk1, v1 = proj_start(1)
        units = [lambda c=c: proj_qk_unit(c, xt1, qk1) for c in range(6)]
        units += [lambda tt=tt: proj_v_unit(tt, xt1, v1) for tt in range(4)]
        for b, qk_b, v_b in ((0, qk0, v0), (1, qk1, v1)):
            for j in range(NH // 2):
                tail = attn_pair_scores(b, j, qk_b, v_b)
                tail()
                if b == 0:
                    for u in units[2 * j : 2 * j + 2]:
                        u()

    nc.compile()
    return nc


_CACHE = {}
_LOCK = threading.Lock()

# test-harness hooks: set TRACE=True before calling kernel() to profile; the
# BassKernelResults of the last run lands in LAST_RESULT.
TRACE = False
LAST_RESULT = None


def _get_nc():
    with _LOCK:
        if "nc" not in _CACHE:
            _CACHE["nc"] = build_program()
        return _CACHE["nc"]


def _numpy_fallback(hidden_state, attention_mask, Wq, bq, Wk, bk, Wv, bv):
    x = hidden_state.astype(np.float32)
    q = (x @ Wq.T + bq).reshape(B, S, NH, HD).transpose(0, 2, 1, 3)
    k = (x @ Wk.T + bk).reshape(B, S, NH, HD).transpose(0, 2, 1, 3)
    v = (x @ Wv.T + bv).reshape(B, S, NH, HD).transpose(0, 2, 1, 3)
    scores = np.einsum("bhqd,bhkd->bhqk", q, k).astype(np.float32) * np.float32(SCALE)
    scores = scores + ((1.0 - attention_mask)[:, None, None, :] * -10000.0).astype(
        np.float32
    )
    m = scores.max(-1, keepdims=True)
    e = np.exp(scores - m, dtype=np.float32)
    p = e / e.sum(-1, keepdims=True, dtype=np.float32)
    ctx = np.einsum("bhqk,bhkd->bhqd", p, v).astype(np.float32)
    ctx = ctx.transpose(0, 2, 1, 3).reshape(B, S, NH * HD)
    return ctx.astype(np.float32), p.astype(np.float32)


def kernel(hidden_state, attention_mask, Wq, bq, Wk, bk, Wv, bv):
    hidden_state = np.ascontiguousarray(np.asarray(hidden_state, dtype=np.float32))
    attention_mask = np.asarray(attention_mask, dtype=np.float32)
    Wq = np.asarray(Wq, dtype=np.float32)
    Wk = np.asarray(Wk, dtype=np.float32)
    Wv = np.asarray(Wv, dtype=np.float32)
    bq = np.asarray(bq, dtype=np.float32)
    bk = np.asarray(bk, dtype=np.float32)
    bv = np.asarray(bv, dtype=np.float32)

    # The device program folds the (all-ones per the problem spec) attention
    # mask out entirely; handle the general case exactly on the host.
    if not np.all(attention_mask == 1.0):
        return _numpy_fallback(hidden_state, attention_mask, Wq, bq, Wk, bk, Wv, bv)

    # Interleave Wq/Wk in 128-row blocks so each projection output M-tile
    # holds [Q^T head pair; K^T head pair] on matching partition ranges.
    wqk = np.empty((2 * H, H), dtype=np.float32)
    bqk = np.empty((2 * H,), dtype=np.float32)
    for jj in range(6):
        wqk[256 * jj : 256 * jj + 128] = Wq[128 * jj : 128 * jj + 128]
        wqk[256 * jj + 128 : 256 * jj + 256] = Wk[128 * jj : 128 * jj + 128]
        bqk[256 * jj : 256 * jj + 128] = bq[128 * jj : 128 * jj + 128]
        bqk[256 * jj + 128 : 256 * jj + 256] = bk[128 * jj : 128 * jj + 128]
    wqkt = np.ascontiguousarray(wqk.T)
    wvt = np.ascontiguousarray(Wv.T)

    nc = _get_nc()
    in_maps = []
    for c in range(NCORES):
        xs = hidden_state[BPC * c : BPC * c + BPC]
        in_maps.append(
            {
                "xt": np.ascontiguousarray(xs.transpose(0, 2, 1)),
                "wqkt": wqkt,
                "wvt": wvt,
                "bqk": bqk,
                "bv": bv,
            }
        )
    res = run_bass_kernel_spmd(nc, in_maps, core_ids=list(range(NCORES)), trace=TRACE)
    global LAST_RESULT
    LAST_RESULT = res
    ctx = np.concatenate([r["ctxo"] for r in res.results], axis=0)
    probs = np.concatenate([r["probs"] for r in res.results], axis=0)
    return ctx.astype(np.float32, copy=False), probs.astype(np.float32, copy=False)
